# revision 4
# baseline (speedup 1.0000x reference)
"""BlendShapes model kernel for 8 Trainium2 NeuronCores (v2: warm-PE design).

Computation (reference):
    pose_repr = pose[:, 1:].reshape(B, 23, 9) - eye      # (B, J, 9)
    per-joint MLP 9 -> 18 -> 32 -> 8 (ReLU between)      # coff (B, J, 8)
    basis_full = basis[:, None] * mask[:, :, None, None]  # (V, J, 8, 3)
    res = einsum('bjk,vjkc->bvc', coff, basis_full)       # (B, V, 3)

Mapping (per core; vertices sharded 8 ways, V=6890 padded to 8*864):
  - Host precomputes bfm = basis*mask*2^13 (f16, rows = (j,k), cols = (v,c))
    and folds the eye-subtraction into the L1 bias (b1' = b1 - e @ W1), so
    the device does only matmuls + epilogues + the output store.
  - The PE's HAM clock gate runs matmuls at 1.2 GHz until ~3.4us of sustained
    activity, then 2.4 GHz. A short warm-up matmul stream runs during the
    input DMAs so the real work executes warm.
  - MLP joint chunks of 4 (6 chunks):
      L1 (K=36, M=72):  chunk pairs row-tiled at PE rows 0 / 64 -> 2x
      L2 (K=72, M=128): plain matmuls
      L3 (K=128, M=32): 4 chunks col-tiled into one PSUM tile -> the
        coefficients land directly in coffT layout (no merge DMAs)
  - Main GEMM out[b, (v,c)] = coffT.T @ bfm, K=184 split 128+56, per b-tile
    stationary reuse across six 512-wide N-tiles (LDWEIGHTS hidden).
  - Output stored f16 (descale 2^-13 folded into the PSUM evacuation);
    host converts to f32.
"""

import numpy as np

N_VERT, N_JOINT, BPJ, BATCH = 6890, 23, 8, 1024
VPAD = 6912  # 8 * 864
VC = VPAD // 8  # 864 vertices per core
VC3 = VC * 3  # 2592
NB = BATCH // 128  # 8 b-tiles
NT_BOUNDS = [0, 512, 1024, 1536, 2048, 2560, 2592]

CHUNKS = [(0, 4), (4, 8), (8, 12), (12, 16), (16, 20), (20, 23)]
NCH = len(CHUNKS)


def _offsets(mpj):
    offs, col = [], 0
    for js, je in CHUNKS:
        offs.append(col)
        col += (je - js) * mpj
    return offs, col


W1_OFF, W1_TOT = _offsets(18)  # 414
W2_OFF, W2_TOT = _offsets(32)  # 736
W3_OFF, W3_TOT = _offsets(8)   # 184
W2_OFF = [W1_TOT + o for o in W2_OFF]
W3_OFF = [W1_TOT + W2_TOT + o for o in W3_OFF]
W_COLS = W1_TOT + W2_TOT + W3_TOT  # 1334

# bias_all columns: [0:6] L1 bias (eye-folded), [6:12] L2 bias,
# [12] L3 bias stacked for coffT_a (128 rows), [13] same for coffT_b (56).
BIAS_COLS = 14
BSCALE = 8192.0  # 2**13, exact in f16/f32
DESCALE = 1.0 / 8192.0
N_WARMUP = 9  # warm-up matmuls (N=512) to flip the HAM clock gate

_CACHED = {}


def _build_nc():
    import concourse.tile as tile
    from concourse import bacc, mybir
    from contextlib import ExitStack

    dt = mybir.dt
    f32, f16 = dt.float32, dt.float16
    AF = mybir.ActivationFunctionType
    ALU = mybir.AluOpType

    nc = bacc.Bacc(None, target_bir_lowering=False)

    pose_t = nc.dram_tensor("pose_t", [207, BATCH], f16, kind="ExternalInput")
    bfm_a_d = nc.dram_tensor("bfm_a", [128, VC3], f16, kind="ExternalInput")
    bfm_b_d = nc.dram_tensor("bfm_b", [56, VC3], f16, kind="ExternalInput")
    w_all = nc.dram_tensor("w_all", [128, W_COLS], f16, kind="ExternalInput")
    bias_all = nc.dram_tensor("bias_all", [128, BIAS_COLS], f32, kind="ExternalInput")
    res = nc.dram_tensor("res", [BATCH, VC3], f16, kind="ExternalOutput")

    with ExitStack() as ctx:
        tc = ctx.enter_context(tile.TileContext(nc))
        const = ctx.enter_context(tc.tile_pool(name="const", bufs=1))
        work = ctx.enter_context(tc.tile_pool(name="work", bufs=1))
        outp = ctx.enter_context(tc.tile_pool(name="outp", bufs=2))
        psum = ctx.enter_context(tc.tile_pool(name="psum", bufs=8, space="PSUM"))

        # ---- input DMAs. sync queue: bias, pose pair 0, w, pose pairs 1-2
        # (critical-path order for the MLP). gpsimd queue: bfm (needed only
        # by the main GEMM; transfers overlap the sync queue's).
        bias_sb = const.tile([128, BIAS_COLS], f32, tag="bias")
        nc.sync.dma_start(out=bias_sb[:], in_=bias_all[:, :])

        # pose tiles: pair p holds chunk 2p at rows 0.. and chunk 2p+1 at
        # rows 64.. (row-tiled L1 needs the odd chunk at PE row base 64).
        pose_p = [work.tile([128, BATCH], f16, tag=f"pose_{p}", name=f"pose_{p}")
                  for p in range(3)]

        def pose_dma(c):
            js, je = CHUNKS[c]
            K = 9 * (je - js)
            p, hi = divmod(c, 2)
            r0 = 64 if hi else 0
            nc.sync.dma_start(
                out=pose_p[p][r0 : r0 + K, :], in_=pose_t[9 * js : 9 * js + K, :]
            )

        pose_dma(0)
        pose_dma(1)
        w_sb = const.tile([128, W_COLS], f16, tag="w")
        nc.sync.dma_start(out=w_sb[:], in_=w_all[:, :])
        for c in range(2, 6):
            pose_dma(c)

        bfm_a = work.tile([128, VC3], f16, tag="bfm_a")
        bfm_b = work.tile([56, VC3], f16, tag="bfm_b")
        nc.gpsimd.dma_start(out=bfm_a[:], in_=bfm_a_d[:, :])
        nc.gpsimd.dma_start(out=bfm_b[:], in_=bfm_b_d[:, :])

        # ---- PE warm-up: matmuls on a zeroed tile while inputs stream in.
        warm = work.tile([128, 512], f16, tag="warm")
        nc.gpsimd.memset(warm[:], 0.0)
        wps = psum.tile([128, 512], f32, tag="ps", name="warm_ps")
        for i in range(N_WARMUP):
            nc.tensor.matmul(
                wps[:], lhsT=warm[:, 0:128], rhs=warm[:], start=True, stop=True
            )

        # ---- MLP ----
        ep_ctr = [0]

        def epilogue(dst, src, bias_ap, relu, scale=None):
            # alternate ACT / DVE so the PE never waits on PSUM (GPSIMD
            # cannot read PSUM; ACT's Copy can't take an AP bias, so
            # bias-only epilogues go to DVE).
            e = ep_ctr[0] % 2
            ep_ctr[0] += 1
            if e == 0 and scale is not None:
                nc.scalar.activation(dst, src, AF.Copy, scale=scale)
            elif e == 0 and relu:
                nc.scalar.activation(dst, src, AF.Relu, bias=bias_ap)
            elif scale is not None:
                nc.vector.tensor_scalar(
                    out=dst, in0=src, scalar1=scale, scalar2=None, op0=ALU.mult
                )
            elif relu:
                nc.vector.tensor_scalar(
                    out=dst, in0=src, scalar1=bias_ap, scalar2=0.0,
                    op0=ALU.add, op1=ALU.max,
                )
            else:
                nc.vector.tensor_scalar(
                    out=dst, in0=src, scalar1=bias_ap, scalar2=None, op0=ALU.add
                )

        h1 = {}
        h2 = {}
        HALves = (slice(0, 512), slice(512, 1024))

        # L1: row-tiled chunk pairs, both halves of B.
        for p in range(3):
            c0, c1 = 2 * p, 2 * p + 1
            K0, M0 = 9 * (CHUNKS[c0][1] - CHUNKS[c0][0]), 18 * (CHUNKS[c0][1] - CHUNKS[c0][0])
            K1, M1 = 9 * (CHUNKS[c1][1] - CHUNKS[c1][0]), 18 * (CHUNKS[c1][1] - CHUNKS[c1][0])
            h1[c0] = work.tile([M0, BATCH], f16, tag=f"h1_{c0}", name=f"h1_{c0}")
            h1[c1] = work.tile([M1, BATCH], f16, tag=f"h1_{c1}", name=f"h1_{c1}")
            for h, hs in enumerate(HALves):
                ps0 = psum.tile([128, 512], f32, tag="ps", name=f"ps1_{c0}_{h}")
                ps1 = psum.tile([128, 512], f32, tag="ps", name=f"ps1_{c1}_{h}")
                nc.tensor.matmul(
                    ps0[0:M0, :], lhsT=w_sb[0:K0, W1_OFF[c0] : W1_OFF[c0] + M0],
                    rhs=pose_p[p][0:K0, hs], start=True, stop=True,
                    tile_position=(0, 0),
                )
                nc.tensor.matmul(
                    ps1[0:M1, :], lhsT=w_sb[64 : 64 + K1, W1_OFF[c1] : W1_OFF[c1] + M1],
                    rhs=pose_p[p][64 : 64 + K1, hs], start=True, stop=True,
                    tile_position=(64, 0),
                )
                epilogue(h1[c0][:, hs], ps0[0:M0, :], bias_sb[0:M0, c0 : c0 + 1], True)
                epilogue(h1[c1][:, hs], ps1[0:M1, :], bias_sb[0:M1, c1 : c1 + 1], True)

        # L2: plain per-chunk matmuls.
        for c, (js, je) in enumerate(CHUNKS):
            nj = je - js
            K, M = 18 * nj, 32 * nj
            h2[c] = work.tile([M, BATCH], f16, tag=f"h2_{c}", name=f"h2_{c}")
            for h, hs in enumerate(HALves):
                ps = psum.tile([128, 512], f32, tag="ps", name=f"ps2_{c}_{h}")
                nc.tensor.matmul(
                    ps[0:M, :], lhsT=w_sb[0:K, W2_OFF[c] : W2_OFF[c] + M],
                    rhs=h1[c][:, hs], start=True, stop=True,
                )
                epilogue(h2[c][:, hs], ps[0:M, :], bias_sb[0:M, 6 + c : 7 + c], True)

        # L3: col-tiled into coffT layout. Group A: chunks 0-3 -> coffT_a
        # partitions 32c..32c+31; group B: chunks 4,5 -> coffT_b 0-55.
        coffT_a = work.tile([128, BATCH], f16, tag="coffT_a")
        coffT_b = work.tile([56, BATCH], f16, tag="coffT_b")
        for h, hs in enumerate(HALves):
            psA = psum.tile([128, 512], f32, tag="ps", name=f"ps3a_{h}")
            for c in range(4):
                nc.tensor.matmul(
                    psA[32 * c : 32 * c + 32, :],
                    lhsT=w_sb[0:128, W3_OFF[c] : W3_OFF[c] + 32],
                    rhs=h2[c][:, hs], start=True, stop=True,
                    tile_position=(0, 32 * c),
                )
            epilogue(coffT_a[:, hs], psA[:, :], bias_sb[0:128, 12:13], False)
            psB = psum.tile([128, 512], f32, tag="ps", name=f"ps3b_{h}")
            nc.tensor.matmul(
                psB[0:32, :], lhsT=w_sb[0:128, W3_OFF[4] : W3_OFF[4] + 32],
                rhs=h2[4][:, hs], start=True, stop=True, tile_position=(0, 0),
            )
            nc.tensor.matmul(
                psB[32:56, :], lhsT=w_sb[0:96, W3_OFF[5] : W3_OFF[5] + 24],
                rhs=h2[5][:, hs], start=True, stop=True, tile_position=(0, 32),
            )
            epilogue(coffT_b[:, hs], psB[0:56, :], bias_sb[0:56, 13:14], False)

        # ---- main GEMM: per b-tile, A-pass (K=128) then B-pass (K=56)
        # accumulating across six N-tiles; evacuate with the 2^-13 descale
        # to f16 and stream out.
        for bt in range(NB):
            bsl = slice(bt * 128, (bt + 1) * 128)
            ostrip = outp.tile([128, VC3], f16, tag="ostrip", name=f"ostrip_{bt}")
            tiles = []
            for t in range(6):
                n0, n1 = NT_BOUNDS[t], NT_BOUNDS[t + 1]
                ps = psum.tile([128, 512], f32, tag="ps", name=f"psm_{bt}_{t}")
                tiles.append(ps)
                nc.tensor.matmul(
                    ps[:, 0 : n1 - n0], lhsT=coffT_a[:, bsl], rhs=bfm_a[:, n0:n1],
                    start=True, stop=False,
                )
            for t in range(6):
                n0, n1 = NT_BOUNDS[t], NT_BOUNDS[t + 1]
                nc.tensor.matmul(
                    tiles[t][:, 0 : n1 - n0], lhsT=coffT_b[:, bsl],
                    rhs=bfm_b[:, n0:n1], start=False, stop=True,
                )
            for t in range(6):
                n0, n1 = NT_BOUNDS[t], NT_BOUNDS[t + 1]
                epilogue(ostrip[:, n0:n1], tiles[t][:, 0 : n1 - n0], None, False,
                         scale=DESCALE)
            nc.sync.dma_start(out=res[bsl, :], in_=ostrip[:])

    nc.finalize()
    return nc


def _pack_host(pose, basis, mask, w1, b1, w2, b2, w3, b3):
    pose_t = np.ascontiguousarray(
        pose[:, 1:].reshape(BATCH, 207).T.astype(np.float16)
    )  # [207, B], rows (j, i)

    # bfm rows (j, k) scaled by 2^13, cols (v, c) padded to VPAD.
    bfm = np.zeros((N_JOINT * BPJ, VPAD * 3), np.float16)
    prod = (basis[:, None, :, :] * mask[:, :, None, None] * BSCALE)  # (V, J, K, 3)
    bfm[:, : N_VERT * 3] = (
        prod.transpose(1, 2, 0, 3).reshape(N_JOINT * BPJ, N_VERT * 3)
    ).astype(np.float16)

    w_all = np.zeros((128, W_COLS), np.float16)
    bias_all = np.zeros((128, BIAS_COLS), np.float32)
    eye9 = np.eye(3, dtype=np.float64).reshape(-1)
    for c, ((js, je), o1, o2, o3) in enumerate(zip(CHUNKS, W1_OFF, W2_OFF, W3_OFF)):
        r1 = 64 if c % 2 else 0  # odd chunks' W1 blocks live at PE rows 64+
        for t, j in enumerate(range(js, je)):
            w_all[r1 + t * 9 : r1 + (t + 1) * 9, o1 + t * 18 : o1 + (t + 1) * 18] = w1[j]
            w_all[t * 18 : (t + 1) * 18, o2 + t * 32 : o2 + (t + 1) * 32] = w2[j]
            w_all[t * 32 : (t + 1) * 32, o3 + t * 8 : o3 + (t + 1) * 8] = w3[j]
    b1f = b1.astype(np.float64) - np.einsum("i,jio->jo", eye9, w1.astype(np.float64))
    for c, (js, je) in enumerate(CHUNKS):
        nj = je - js
        bias_all[0 : 18 * nj, c] = b1f[js:je].reshape(-1).astype(np.float32)
        bias_all[0 : 32 * nj, 6 + c] = b2[js:je].reshape(-1)
    bias_all[0:128, 12] = b3[0:16].reshape(-1)  # chunks 0-3 stacked (4*32)
    bias_all[0:56, 13] = b3[16:23].reshape(-1)  # chunks 4,5 stacked (32+24)

    return pose_t, bfm, w_all, bias_all


def _in_maps(pose, basis, mask, w1, b1, w2, b2, w3, b3):
    pose_t, bfm, w_all, bias_all = _pack_host(
        np.asarray(pose, np.float32),
        np.asarray(basis, np.float32),
        np.asarray(mask, np.float32),
        np.asarray(w1, np.float32),
        np.asarray(b1, np.float32),
        np.asarray(w2, np.float32),
        np.asarray(b2, np.float32),
        np.asarray(w3, np.float32),
        np.asarray(b3, np.float32),
    )
    maps = []
    for i in range(8):
        c0 = i * VC3
        maps.append(
            {
                "pose_t": pose_t,
                "bfm_a": np.ascontiguousarray(bfm[0:128, c0 : c0 + VC3]),
                "bfm_b": np.ascontiguousarray(bfm[128:184, c0 : c0 + VC3]),
                "w_all": w_all,
                "bias_all": bias_all,
            }
        )
    return maps


def kernel(pose, basis, mask, w1, b1, w2, b2, w3, b3):
    from concourse.bass_utils import run_bass_kernel_spmd

    if "nc" not in _CACHED:
        _CACHED["nc"] = _build_nc()
    nc = _CACHED["nc"]

    maps = _in_maps(pose, basis, mask, w1, b1, w2, b2, w3, b3)
    r = run_bass_kernel_spmd(nc, maps, core_ids=list(range(8)))
    out = np.concatenate(
        [m["res"].astype(np.float32).reshape(BATCH, VC, 3) for m in r.results],
        axis=1,
    )
    return np.ascontiguousarray(out[:, :N_VERT, :])


# revision 7
# speedup vs baseline: 1.0138x; 1.0138x over previous
"""BlendShapes model kernel for 8 Trainium2 NeuronCores (v2: warm-PE design).

Computation (reference):
    pose_repr = pose[:, 1:].reshape(B, 23, 9) - eye      # (B, J, 9)
    per-joint MLP 9 -> 18 -> 32 -> 8 (ReLU between)      # coff (B, J, 8)
    basis_full = basis[:, None] * mask[:, :, None, None]  # (V, J, 8, 3)
    res = einsum('bjk,vjkc->bvc', coff, basis_full)       # (B, V, 3)

Mapping (per core; vertices sharded 8 ways, V=6890 padded to 8*864):
  - Host precomputes bfm = basis*mask*2^13 (f16, rows = (j,k), cols = (v,c))
    and folds the eye-subtraction into the L1 bias (b1' = b1 - e @ W1), so
    the device does only matmuls + epilogues + the output store.
  - The PE's HAM clock gate runs matmuls at 1.2 GHz until ~3.4us of sustained
    activity, then 2.4 GHz. A short warm-up matmul stream runs during the
    input DMAs so the real work executes warm.
  - MLP joint chunks of 4 (6 chunks):
      L1 (K=36, M=72):  chunk pairs row-tiled at PE rows 0 / 64 -> 2x
      L2 (K=72, M=128): plain matmuls
      L3 (K=128, M=32): 4 chunks col-tiled into one PSUM tile -> the
        coefficients land directly in coffT layout (no merge DMAs)
  - Main GEMM out[b, (v,c)] = coffT.T @ bfm, K=184 split 128+56, per b-tile
    stationary reuse across six 512-wide N-tiles (LDWEIGHTS hidden).
  - Output stored f16 (descale 2^-13 folded into the PSUM evacuation);
    host converts to f32.
"""

import numpy as np

N_VERT, N_JOINT, BPJ, BATCH = 6890, 23, 8, 1024
VPAD = 6912  # 8 * 864
VC = VPAD // 8  # 864 vertices per core
VC3 = VC * 3  # 2592
NB = BATCH // 128  # 8 b-tiles
NT_BOUNDS = [0, 512, 1024, 1536, 2048, 2560, 2592]

CHUNKS = [(0, 4), (4, 8), (8, 12), (12, 16), (16, 20), (20, 23)]
NCH = len(CHUNKS)


def _offsets(mpj):
    offs, col = [], 0
    for js, je in CHUNKS:
        offs.append(col)
        col += (je - js) * mpj
    return offs, col


W1_OFF, W1_TOT = _offsets(18)  # 414
W2_OFF, W2_TOT = _offsets(32)  # 736
W3_OFF, W3_TOT = _offsets(8)   # 184
W2_OFF = [W1_TOT + o for o in W2_OFF]
W3_OFF = [W1_TOT + W2_TOT + o for o in W3_OFF]
W_COLS = W1_TOT + W2_TOT + W3_TOT  # 1334

# bias_all columns: [0:6] L1 bias (eye-folded), [6:12] L2 bias,
# [12] L3 bias stacked for coffT_a (128 rows), [13] same for coffT_b (56).
BIAS_COLS = 14
BSCALE = 8192.0  # 2**13, exact in f16/f32
DESCALE = 1.0 / 8192.0
N_WARMUP = 7  # warm-up matmuls (N=512) to flip the HAM clock gate

_CACHED = {}


def _build_nc():
    import concourse.tile as tile
    from concourse import bacc, mybir
    from contextlib import ExitStack

    dt = mybir.dt
    f32, f16 = dt.float32, dt.float16
    AF = mybir.ActivationFunctionType
    ALU = mybir.AluOpType

    nc = bacc.Bacc(None, target_bir_lowering=False)

    pose_t = nc.dram_tensor("pose_t", [207, BATCH], f16, kind="ExternalInput")
    bfm_a_d = nc.dram_tensor("bfm_a", [128, VC3], f16, kind="ExternalInput")
    bfm_b_d = nc.dram_tensor("bfm_b", [56, VC3], f16, kind="ExternalInput")
    w_all = nc.dram_tensor("w_all", [128, W_COLS], f16, kind="ExternalInput")
    bias_all = nc.dram_tensor("bias_all", [128, BIAS_COLS], f32, kind="ExternalInput")
    res = nc.dram_tensor("res", [BATCH, VC3], f16, kind="ExternalOutput")

    with ExitStack() as ctx:
        tc = ctx.enter_context(tile.TileContext(nc))
        const = ctx.enter_context(tc.tile_pool(name="const", bufs=1))
        work = ctx.enter_context(tc.tile_pool(name="work", bufs=1))
        outp = ctx.enter_context(tc.tile_pool(name="outp", bufs=2))
        psum = ctx.enter_context(tc.tile_pool(name="psum", bufs=8, space="PSUM"))

        # ---- PE warm-up source: memset on DVE (its queue opens early) so
        # the warm-up matmuls start the moment the tensor queue opens.
        warm = work.tile([128, 512], f16, tag="warm")
        nc.vector.memset(warm[:], 0.0)

        # ---- input DMAs. sync queue: w + pose in MLP critical-path order.
        # gpsimd queue: bias + bfm (bias is tiny; bfm is needed only by the
        # main GEMM and its transfers overlap the sync queue's).
        w_sb = const.tile([128, W_COLS], f16, tag="w")
        nc.sync.dma_start(out=w_sb[:], in_=w_all[:, :])

        # pose tiles: pair p holds chunk 2p at rows 0.. and chunk 2p+1 at
        # rows 64.. (row-tiled L1 needs the odd chunk at PE row base 64).
        pose_p = [work.tile([128, BATCH], f16, tag=f"pose_{p}", name=f"pose_{p}")
                  for p in range(3)]

        def pose_dma(c):
            js, je = CHUNKS[c]
            K = 9 * (je - js)
            p, hi = divmod(c, 2)
            r0 = 64 if hi else 0
            nc.sync.dma_start(
                out=pose_p[p][r0 : r0 + K, :], in_=pose_t[9 * js : 9 * js + K, :]
            )

        for c in range(6):
            pose_dma(c)

        bias_sb = const.tile([128, BIAS_COLS], f32, tag="bias")
        nc.gpsimd.dma_start(out=bias_sb[:], in_=bias_all[:, :])
        bfm_a = work.tile([128, VC3], f16, tag="bfm_a")
        bfm_b = work.tile([56, VC3], f16, tag="bfm_b")
        nc.gpsimd.dma_start(out=bfm_a[:], in_=bfm_a_d[:, :])
        nc.gpsimd.dma_start(out=bfm_b[:], in_=bfm_b_d[:, :])

        # ---- PE warm-up: matmuls on the zeroed tile while inputs stream in
        # (the HAM clock gate needs ~6us of sustained PE activity to release
        # the 1.2 GHz throttle).
        wps = psum.tile([128, 512], f32, tag="ps", name="warm_ps")
        for i in range(N_WARMUP):
            nc.tensor.matmul(
                wps[:], lhsT=warm[:, 0:128], rhs=warm[:], start=True, stop=True
            )

        # ---- MLP ----
        ep_ctr = [0]

        def epilogue(dst, src, bias_ap, relu, scale=None):
            # alternate ACT / DVE so the PE never waits on PSUM (GPSIMD
            # cannot read PSUM; ACT's Copy can't take an AP bias, so
            # bias-only epilogues go to DVE).
            e = ep_ctr[0] % 2
            ep_ctr[0] += 1
            if e == 0 and scale is not None:
                nc.scalar.activation(dst, src, AF.Copy, scale=scale)
            elif e == 0 and relu:
                nc.scalar.activation(dst, src, AF.Relu, bias=bias_ap)
            elif scale is not None:
                nc.vector.tensor_scalar(
                    out=dst, in0=src, scalar1=scale, scalar2=None, op0=ALU.mult
                )
            elif relu:
                nc.vector.tensor_scalar(
                    out=dst, in0=src, scalar1=bias_ap, scalar2=0.0,
                    op0=ALU.add, op1=ALU.max,
                )
            else:
                nc.vector.tensor_scalar(
                    out=dst, in0=src, scalar1=bias_ap, scalar2=None, op0=ALU.add
                )

        h1 = {}
        h2 = {}
        HALves = (slice(0, 512), slice(512, 1024))

        # L1: row-tiled chunk pairs, both halves of B.
        for p in range(3):
            c0, c1 = 2 * p, 2 * p + 1
            K0, M0 = 9 * (CHUNKS[c0][1] - CHUNKS[c0][0]), 18 * (CHUNKS[c0][1] - CHUNKS[c0][0])
            K1, M1 = 9 * (CHUNKS[c1][1] - CHUNKS[c1][0]), 18 * (CHUNKS[c1][1] - CHUNKS[c1][0])
            h1[c0] = work.tile([M0, BATCH], f16, tag=f"h1_{c0}", name=f"h1_{c0}")
            h1[c1] = work.tile([M1, BATCH], f16, tag=f"h1_{c1}", name=f"h1_{c1}")
            for h, hs in enumerate(HALves):
                ps0 = psum.tile([128, 512], f32, tag="ps", name=f"ps1_{c0}_{h}")
                ps1 = psum.tile([128, 512], f32, tag="ps", name=f"ps1_{c1}_{h}")
                nc.tensor.matmul(
                    ps0[0:M0, :], lhsT=w_sb[0:K0, W1_OFF[c0] : W1_OFF[c0] + M0],
                    rhs=pose_p[p][0:K0, hs], start=True, stop=True,
                    tile_position=(0, 0),
                )
                nc.tensor.matmul(
                    ps1[0:M1, :], lhsT=w_sb[64 : 64 + K1, W1_OFF[c1] : W1_OFF[c1] + M1],
                    rhs=pose_p[p][64 : 64 + K1, hs], start=True, stop=True,
                    tile_position=(64, 0),
                )
                epilogue(h1[c0][:, hs], ps0[0:M0, :], bias_sb[0:M0, c0 : c0 + 1], True)
                epilogue(h1[c1][:, hs], ps1[0:M1, :], bias_sb[0:M1, c1 : c1 + 1], True)

        # L2: plain per-chunk matmuls.
        for c, (js, je) in enumerate(CHUNKS):
            nj = je - js
            K, M = 18 * nj, 32 * nj
            h2[c] = work.tile([M, BATCH], f16, tag=f"h2_{c}", name=f"h2_{c}")
            for h, hs in enumerate(HALves):
                ps = psum.tile([128, 512], f32, tag="ps", name=f"ps2_{c}_{h}")
                nc.tensor.matmul(
                    ps[0:M, :], lhsT=w_sb[0:K, W2_OFF[c] : W2_OFF[c] + M],
                    rhs=h1[c][:, hs], start=True, stop=True,
                )
                epilogue(h2[c][:, hs], ps[0:M, :], bias_sb[0:M, 6 + c : 7 + c], True)

        # L3: col-tiled into coffT layout. Group A: chunks 0-3 -> coffT_a
        # partitions 32c..32c+31; group B: chunks 4,5 -> coffT_b 0-55.
        coffT_a = work.tile([128, BATCH], f16, tag="coffT_a")
        coffT_b = work.tile([56, BATCH], f16, tag="coffT_b")
        for h, hs in enumerate(HALves):
            psA = psum.tile([128, 512], f32, tag="ps", name=f"ps3a_{h}")
            for c in range(4):
                nc.tensor.matmul(
                    psA[32 * c : 32 * c + 32, :],
                    lhsT=w_sb[0:128, W3_OFF[c] : W3_OFF[c] + 32],
                    rhs=h2[c][:, hs], start=True, stop=True,
                    tile_position=(0, 32 * c),
                )
            epilogue(coffT_a[:, hs], psA[:, :], bias_sb[0:128, 12:13], False)
            psB = psum.tile([128, 512], f32, tag="ps", name=f"ps3b_{h}")
            nc.tensor.matmul(
                psB[0:32, :], lhsT=w_sb[0:128, W3_OFF[4] : W3_OFF[4] + 32],
                rhs=h2[4][:, hs], start=True, stop=True, tile_position=(0, 0),
            )
            nc.tensor.matmul(
                psB[32:56, :], lhsT=w_sb[0:96, W3_OFF[5] : W3_OFF[5] + 24],
                rhs=h2[5][:, hs], start=True, stop=True, tile_position=(0, 32),
            )
            epilogue(coffT_b[:, hs], psB[0:56, :], bias_sb[0:56, 13:14], False)

        # ---- main GEMM: per b-tile, A-pass (K=128) then B-pass (K=56)
        # accumulating across six N-tiles; evacuate with the 2^-13 descale
        # to f16 and stream out.
        for bt in range(NB):
            bsl = slice(bt * 128, (bt + 1) * 128)
            ostrip = outp.tile([128, VC3], f16, tag="ostrip", name=f"ostrip_{bt}")
            tiles = []
            for t in range(6):
                n0, n1 = NT_BOUNDS[t], NT_BOUNDS[t + 1]
                ps = psum.tile([128, 512], f32, tag="ps", name=f"psm_{bt}_{t}")
                tiles.append(ps)
                nc.tensor.matmul(
                    ps[:, 0 : n1 - n0], lhsT=coffT_a[:, bsl], rhs=bfm_a[:, n0:n1],
                    start=True, stop=False,
                )
            for t in range(6):
                n0, n1 = NT_BOUNDS[t], NT_BOUNDS[t + 1]
                nc.tensor.matmul(
                    tiles[t][:, 0 : n1 - n0], lhsT=coffT_b[:, bsl],
                    rhs=bfm_b[:, n0:n1], start=False, stop=True,
                )
            for t in range(6):
                n0, n1 = NT_BOUNDS[t], NT_BOUNDS[t + 1]
                epilogue(ostrip[:, n0:n1], tiles[t][:, 0 : n1 - n0], None, False,
                         scale=DESCALE)
                if t % 2 == 1:
                    # stream out per pair of evacuated tiles so the store
                    # overlaps the next tiles' evacuation (kills the tail).
                    d0, d1 = NT_BOUNDS[t - 1], n1
                    nc.sync.dma_start(
                        out=res[bsl, d0:d1], in_=ostrip[:, d0:d1]
                    )

    nc.finalize()
    return nc


def _pack_host(pose, basis, mask, w1, b1, w2, b2, w3, b3):
    pose_t = np.ascontiguousarray(
        pose[:, 1:].reshape(BATCH, 207).T.astype(np.float16)
    )  # [207, B], rows (j, i)

    # bfm rows (j, k) scaled by 2^13, cols (v, c) padded to VPAD.
    bfm = np.zeros((N_JOINT * BPJ, VPAD * 3), np.float16)
    prod = (basis[:, None, :, :] * mask[:, :, None, None] * BSCALE)  # (V, J, K, 3)
    bfm[:, : N_VERT * 3] = (
        prod.transpose(1, 2, 0, 3).reshape(N_JOINT * BPJ, N_VERT * 3)
    ).astype(np.float16)

    w_all = np.zeros((128, W_COLS), np.float16)
    bias_all = np.zeros((128, BIAS_COLS), np.float32)
    eye9 = np.eye(3, dtype=np.float64).reshape(-1)
    for c, ((js, je), o1, o2, o3) in enumerate(zip(CHUNKS, W1_OFF, W2_OFF, W3_OFF)):
        r1 = 64 if c % 2 else 0  # odd chunks' W1 blocks live at PE rows 64+
        for t, j in enumerate(range(js, je)):
            w_all[r1 + t * 9 : r1 + (t + 1) * 9, o1 + t * 18 : o1 + (t + 1) * 18] = w1[j]
            w_all[t * 18 : (t + 1) * 18, o2 + t * 32 : o2 + (t + 1) * 32] = w2[j]
            w_all[t * 32 : (t + 1) * 32, o3 + t * 8 : o3 + (t + 1) * 8] = w3[j]
    b1f = b1.astype(np.float64) - np.einsum("i,jio->jo", eye9, w1.astype(np.float64))
    for c, (js, je) in enumerate(CHUNKS):
        nj = je - js
        bias_all[0 : 18 * nj, c] = b1f[js:je].reshape(-1).astype(np.float32)
        bias_all[0 : 32 * nj, 6 + c] = b2[js:je].reshape(-1)
    bias_all[0:128, 12] = b3[0:16].reshape(-1)  # chunks 0-3 stacked (4*32)
    bias_all[0:56, 13] = b3[16:23].reshape(-1)  # chunks 4,5 stacked (32+24)

    return pose_t, bfm, w_all, bias_all


def _in_maps(pose, basis, mask, w1, b1, w2, b2, w3, b3):
    pose_t, bfm, w_all, bias_all = _pack_host(
        np.asarray(pose, np.float32),
        np.asarray(basis, np.float32),
        np.asarray(mask, np.float32),
        np.asarray(w1, np.float32),
        np.asarray(b1, np.float32),
        np.asarray(w2, np.float32),
        np.asarray(b2, np.float32),
        np.asarray(w3, np.float32),
        np.asarray(b3, np.float32),
    )
    maps = []
    for i in range(8):
        c0 = i * VC3
        maps.append(
            {
                "pose_t": pose_t,
                "bfm_a": np.ascontiguousarray(bfm[0:128, c0 : c0 + VC3]),
                "bfm_b": np.ascontiguousarray(bfm[128:184, c0 : c0 + VC3]),
                "w_all": w_all,
                "bias_all": bias_all,
            }
        )
    return maps


def kernel(pose, basis, mask, w1, b1, w2, b2, w3, b3):
    from concourse.bass_utils import run_bass_kernel_spmd

    if "nc" not in _CACHED:
        _CACHED["nc"] = _build_nc()
    nc = _CACHED["nc"]

    maps = _in_maps(pose, basis, mask, w1, b1, w2, b2, w3, b3)
    r = run_bass_kernel_spmd(nc, maps, core_ids=list(range(8)))
    out = np.concatenate(
        [m["res"].astype(np.float32).reshape(BATCH, VC, 3) for m in r.results],
        axis=1,
    )
    return np.ascontiguousarray(out[:, :N_VERT, :])
